# revision 6
# baseline (speedup 1.0000x reference)
"""Trainium2 Bass kernel for nn_ConvHDC (binary HDC conv encoder + classifier).

Strategy (per sharding hint): shard the D=10000 hyperdimension across the 8
NeuronCores (pad to 10240 -> 1280 channels/core = 10 tiles of 128 partitions).
Everything after conv1 is depthwise / per-channel, so the whole network runs
locally per core; only the final [16,10] partial similarity matrix is
AllReduce-summed across cores.

Per-core pipeline (per 128-channel tile):
  conv1  : TensorE matmul, patches [25,2705] x sign(W1) [25,128] -> PSUM
           (last rhs column is the column-sum of patches => per-channel SUM
            of conv1 outputs lands in PSUM for free - used for BN mean)
  BN1    : sum from matmul; sum-of-squares via chained tensor_tensor_reduce
           (DVE); binarize folded into one ACT pass: sign(x + bias) with
           bias = (beta-0.5)/gamma*sqrt(var+eps) - mean  (valid for gamma>0)
  conv2  : depthwise 3x3/s2 as 9 accumulating TensorE matmuls with
           diagonalized per-channel weights (built by one gpsimd
           affine_select per tile), bf16 (exact: h1b in {-1,0,1})
  BN2    : bn_stats/bn_aggr (equal-size chunks keep bn_aggr exact)
  conv3  : depthwise 6x6 -> 1x1 on DVE (broadcast multiply + reduce)
  BN3    : bn_stats/bn_aggr
  final  : [16,10] partial sims via 10 accumulating matmuls (bf16, exact),
           scale 1/sqrt(D), DRAM bounce, AllReduce over 8 cores.
"""

import sys

if "/opt/trn_rl_repo" not in sys.path:
    sys.path.insert(0, "/opt/trn_rl_repo")

import numpy as np
from numpy.lib.stride_tricks import sliding_window_view

from concourse import bacc, tile, mybir
from concourse import bass as bass_mod

F32 = mybir.dt.float32
BF16 = mybir.dt.bfloat16
ALU = mybir.AluOpType
ACTF = mybir.ActivationFunctionType

NCORES = 8
D = 10000
DPAD = 10240
DP = DPAD // NCORES          # 1280 channels per core
DT = DP // 128               # 10 tiles of 128 channels
B = 16
EPS = 1e-5

H1 = 13                      # conv1 output spatial
N1 = B * H1 * H1             # 2704
CH1 = [452, 452, 452, 452, 452, 444]   # conv1 chunk sizes (last +1 sum col)
H2 = 6
N2 = B * H2 * H2             # 576
NB2 = 8                      # batches per conv2 group (2 groups of 288 cols)
G2 = N2 // 2                 # 288

# conv1 matmul input dtype: float32 (exact, 4 cy/row) or float32r (1 cy/row,
# precision must be validated on hardware before enabling)
CONV1_DTYPE = "float32"

_CACHE = {}


def _build_bass():
    """Build + compile the 8-core Bass program. Returns the Bacc object."""
    nc = bacc.Bacc("TRN2", target_bir_lowering=False, debug=False,
                   num_devices=NCORES)

    patches_d = nc.dram_tensor("patches", [25, N1], F32, kind="ExternalInput").ap()
    w1t_d = nc.dram_tensor("w1t", [25, DP], F32, kind="ExternalInput").ap()
    w2_d = nc.dram_tensor("w2", [DP, 9], F32, kind="ExternalInput").ap()
    w3_d = nc.dram_tensor("w3", [DP, 36], F32, kind="ExternalInput").ap()
    bn_d = nc.dram_tensor("bn", [DP, 6], F32, kind="ExternalInput").ap()
    wct_d = nc.dram_tensor("wct", [DP, 10], F32, kind="ExternalInput").ap()
    out_d = nc.dram_tensor("sims", [B, 10], F32, kind="ExternalOutput").ap()

    with tile.TileContext(nc) as tc:
        with (
            tc.tile_pool(name="const", bufs=1) as const,
            tc.tile_pool(name="work", bufs=3) as work,
            tc.tile_pool(name="stat", bufs=2) as stat,
            tc.tile_pool(name="psum1", bufs=1, space="PSUM") as psum1,
            tc.tile_pool(name="psum2", bufs=1, space="PSUM") as psum2,
            tc.tile_pool(name="dram", bufs=1, space="DRAM") as dram,
        ):
            # ---------------- constants / weights ----------------
            patches = const.tile([25, N1 + 1], F32)
            nc.sync.dma_start(out=patches[:, 0:N1], in_=patches_d[:])
            # last column = row-sum of patches (feeds BN1 mean via matmul)
            nc.vector.tensor_reduce(
                patches[:, N1:N1 + 1], patches[:, 0:N1], mybir.AxisListType.X,
                ALU.add)

            w1raw = const.tile([25, DP], F32)
            nc.sync.dma_start(out=w1raw[:], in_=w1t_d[:])
            w1s = const.tile([25, DP], F32)
            nc.scalar.sign(w1s[:], w1raw[:])

            w2raw = const.tile([128, DT, 9], F32)
            nc.sync.dma_start(
                out=w2raw[:], in_=w2_d.rearrange("(t p) k -> p t k", t=DT, p=128))
            w2s = const.tile([128, DT, 9], BF16)
            nc.scalar.sign(w2s[:], w2raw[:])

            w3t = const.tile([128, DT, 36], F32)
            nc.sync.dma_start(
                out=w3t[:], in_=w3_d.rearrange("(t p) k -> p t k", t=DT, p=128))

            bnt = const.tile([128, DT, 6], F32)
            nc.sync.dma_start(
                out=bnt[:], in_=bn_d.rearrange("(t p) c -> p t c", t=DT, p=128))

            wcraw = const.tile([128, DT, 10], F32)
            nc.sync.dma_start(
                out=wcraw[:], in_=wct_d.rearrange("(t p) c -> p t c", t=DT, p=128))
            wcs = const.tile([128, DT, 10], BF16)
            nc.scalar.sign(wcs[:], wcraw[:])

            epsc = const.tile([128, 1], F32)
            nc.vector.memset(epsc[:], EPS)

            # beMrg[:, t, L] = (beta_L - 0.5) / gamma_L   (gamma > 0 assumed)
            rg = const.tile([128, DT, 3], F32)
            nc.vector.reciprocal(rg[:], bnt[:, :, 0::2])
            beMrg = const.tile([128, DT, 3], F32)
            nc.vector.tensor_scalar(beMrg[:], bnt[:, :, 1::2], -0.5, None, ALU.add)
            nc.vector.tensor_tensor(beMrg[:], beMrg[:], rg[:], ALU.mult)

            h3b_all = const.tile([128, DT, B], BF16)

            mm_dt = mybir.dt.float32r if CONV1_DTYPE == "float32r" else F32

            # ---------------- per-tile pipeline ----------------
            for t in range(DT):
                lhs1 = w1s[:, t * 128:(t + 1) * 128]
                if CONV1_DTYPE == "float32r":
                    lhs1 = lhs1.bitcast(mm_dt)

                h1b = work.tile([128, N1], BF16, tag="h1b")
                st1 = stat.tile([128, 6, 6], F32, tag="st1")

                p1 = []
                off = 0
                for ci, csz in enumerate(CH1):
                    w = csz + 1 if ci == 5 else csz
                    pt = psum1.tile([128, w], F32, tag=f"c1_{ci}")
                    rhs = patches[:, off:off + w]
                    if CONV1_DTYPE == "float32r":
                        rhs = rhs.bitcast(mm_dt)
                    nc.tensor.matmul(pt[:], lhsT=lhs1, rhs=rhs,
                                     start=True, stop=True)
                    nc.vector.bn_stats(st1[:, ci, :], pt[:, 0:csz])
                    p1.append(pt)
                    off += csz

                # BN1: mean from matmul sum column; var from exact combine of
                # bn_stats sub-group stats (S2 = sum nv_i + sum n_i*m_i^2,
                # group counts are compile-time constants: 226 for the five
                # 452-chunks' even/odd halves, 222 for the last 444-chunk).
                mean1 = stat.tile([128, 1], F32, tag="mean1")
                nc.vector.tensor_scalar(
                    mean1[:], p1[5][:, 444:445], 1.0 / N1, None, ALU.mult)
                m2 = stat.tile([128, 1], F32, tag="m2")
                nc.vector.tensor_tensor(m2[:], mean1[:], mean1[:], ALU.mult)
                mm = stat.tile([128, 6, 2], F32, tag="mm")
                nc.vector.tensor_tensor(
                    mm[:], st1[:, :, 1::3], st1[:, :, 1::3], ALU.mult)
                s226 = stat.tile([128, 1], F32, tag="s226")
                nc.vector.tensor_reduce(
                    s226[:], mm[:, 0:5, :].rearrange("p a b -> p (a b)"),
                    mybir.AxisListType.X, ALU.add)
                s222 = stat.tile([128, 1], F32, tag="s222")
                nc.vector.tensor_reduce(
                    s222[:], mm[:, 5, :], mybir.AxisListType.X, ALU.add)
                snv = stat.tile([128, 1], F32, tag="snv")
                nc.vector.tensor_reduce(
                    snv[:], st1[:, :, 2::3].rearrange("p a b -> p (a b)"),
                    mybir.AxisListType.X, ALU.add)
                u1 = stat.tile([128, 1], F32, tag="u1")
                nc.vector.tensor_scalar(
                    u1[:], s226[:], 226.0, snv[:], ALU.mult, ALU.add)
                ssqt = stat.tile([128, 1], F32, tag="ssqt")
                nc.vector.tensor_scalar(
                    ssqt[:], s222[:], 222.0, u1[:], ALU.mult, ALU.add)
                ve1 = stat.tile([128, 1], F32, tag="ve1")
                nc.vector.tensor_scalar(
                    ve1[:], ssqt[:], 1.0 / N1, m2[:], ALU.mult, ALU.subtract)
                sq1 = stat.tile([128, 1], F32, tag="sq1")
                nc.scalar.activation(sq1[:], ve1[:], ACTF.Sqrt, bias=epsc[:], scale=1.0)
                bias1 = stat.tile([128, 1], F32, tag="bias1")
                nc.vector.tensor_scalar(
                    bias1[:], sq1[:], beMrg[:, t, 0:1], mean1[:],
                    ALU.mult, ALU.subtract)

                off = 0
                for ci, csz in enumerate(CH1):
                    nc.scalar.activation(
                        h1b[:, off:off + csz], p1[ci][:, 0:csz], ACTF.Sign,
                        bias=bias1[:], scale=1.0)
                    off += csz

                # ---- conv2: diagonalized depthwise 3x3 stride 2 ----
                diag9 = work.tile([128, 9, 128], BF16, tag="diag9")
                nc.gpsimd.affine_select(
                    out=diag9[:],
                    in_=w2s[:, t, :].unsqueeze(2).broadcast_to([128, 9, 128]),
                    pattern=[[0, 9], [1, 128]], base=0, channel_multiplier=-1,
                    compare_op=ALU.is_equal, fill=0.0)

                h1b4 = h1b[:].rearrange("p (b h w) -> p b h w", b=B, h=H1, w=H1)
                st2 = stat.tile([128, 2, 6], F32, tag="st2")
                p2 = []
                for g in range(2):
                    pt2 = psum2.tile([128, G2], F32, tag=f"c2_{g}")
                    for k in range(9):
                        kh, kw = k // 3, k % 3
                        rhs = h1b4[:, g * NB2:(g + 1) * NB2,
                                   kh:kh + 11:2, kw:kw + 11:2]
                        nc.tensor.matmul(pt2[:], lhsT=diag9[:, k, :], rhs=rhs,
                                         start=(k == 0), stop=(k == 8))
                    nc.vector.bn_stats(st2[:, g, :], pt2[:])
                    p2.append(pt2)

                mv2 = stat.tile([128, 2], F32, tag="mv2")
                nc.vector.bn_aggr(mv2[:], st2[:])
                sq2 = stat.tile([128, 1], F32, tag="sq2")
                nc.scalar.activation(sq2[:], mv2[:, 1:2], ACTF.Sqrt,
                                     bias=epsc[:], scale=1.0)
                bias2 = stat.tile([128, 1], F32, tag="bias2")
                nc.vector.tensor_scalar(
                    bias2[:], sq2[:], beMrg[:, t, 1:2], mv2[:, 0:1],
                    ALU.mult, ALU.subtract)

                h2b = work.tile([128, N2], F32, tag="h2b")
                for g in range(2):
                    nc.scalar.activation(
                        h2b[:, g * G2:(g + 1) * G2], p2[g][:], ACTF.Sign,
                        bias=bias2[:], scale=1.0)

                # ---- conv3: depthwise 6x6 -> 1 (DVE) ----
                tmp3 = work.tile([128, B, 36], F32, tag="tmp3")
                h2b3 = h2b[:].rearrange("p (b s) -> p b s", b=B, s=36)
                w3b = w3t[:, t, :].unsqueeze(1).broadcast_to([128, B, 36])
                nc.vector.tensor_tensor(tmp3[:], h2b3, w3b, ALU.mult)
                h3pre = work.tile([128, B], F32, tag="h3pre")
                nc.vector.tensor_reduce(h3pre[:], tmp3[:], mybir.AxisListType.X,
                                        ALU.add)

                st3 = stat.tile([128, 6], F32, tag="st3")
                nc.vector.bn_stats(st3[:], h3pre[:])
                mv3 = stat.tile([128, 2], F32, tag="mv3")
                nc.vector.bn_aggr(mv3[:], st3[:])
                sq3 = stat.tile([128, 1], F32, tag="sq3")
                nc.scalar.activation(sq3[:], mv3[:, 1:2], ACTF.Sqrt,
                                     bias=epsc[:], scale=1.0)
                bias3 = stat.tile([128, 1], F32, tag="bias3")
                nc.vector.tensor_scalar(
                    bias3[:], sq3[:], beMrg[:, t, 2:3], mv3[:, 0:1],
                    ALU.mult, ALU.subtract)
                nc.scalar.activation(h3b_all[:, t, :], h3pre[:], ACTF.Sign,
                                     bias=bias3[:], scale=1.0)

            # ---------------- classifier + AllReduce ----------------
            psims = psum2.tile([B, 10], F32, tag="c2_0")
            for t in range(DT):
                nc.tensor.matmul(psims[:], lhsT=h3b_all[:, t, :],
                                 rhs=wcs[:, t, :],
                                 start=(t == 0), stop=(t == DT - 1))
            sims_sb = stat.tile([B, 10], F32, tag="sims_sb")
            nc.scalar.mul(sims_sb[:], psims[:], 1.0 / np.sqrt(np.float32(D)))

            in_bounce = dram.tile([B, 10], F32)
            out_bounce = dram.tile([B, 10], F32)
            nc.sync.dma_start(out=in_bounce[:], in_=sims_sb[:])
            nc.gpsimd.collective_compute(
                "AllReduce", ALU.add,
                replica_groups=[list(range(NCORES))],
                ins=[in_bounce.opt()], outs=[out_bounce.opt()])
            nc.sync.dma_start(out=out_d[:], in_=out_bounce[:])

    nc.compile()
    return nc


def get_nc():
    if "nc" not in _CACHE:
        _CACHE["nc"] = _build_bass()
    return _CACHE["nc"]


def prep_inputs(x, W1, b1, g1, be1, W2, b2, g2, be2, W3, b3, g3, be3, Wc):
    """Host-side layout/sharding prep (layout only, no model math).

    Conv biases b1/b2/b3 are dropped: training-mode BN is invariant to a
    per-channel additive constant before normalization.
    """
    f = np.float32

    xp = np.zeros((B, 30, 30), f)
    xp[:, 1:29, 1:29] = np.asarray(x, f)[:, 0]
    win = sliding_window_view(xp, (5, 5), axis=(1, 2))[:, ::2, ::2]  # B,13,13,5,5
    patches = np.ascontiguousarray(
        win.transpose(3, 4, 0, 1, 2).reshape(25, N1)).astype(f)

    def padrows(a, width, fill=0.0):
        out = np.full((DPAD, width), fill, f)
        out[:D] = np.asarray(a, f).reshape(D, width)
        return out

    w1p = padrows(W1, 25)               # [DPAD, 25]
    w2p = padrows(W2, 9)
    w3p = padrows(W3, 36)
    wcp = padrows(np.asarray(Wc, f).T, 10)
    bn = np.zeros((DPAD, 6), f)
    bn[:, 0::2] = 1.0                    # pad gamma=1
    bn[:D, 0] = np.asarray(g1, f)
    bn[:D, 1] = np.asarray(be1, f)
    bn[:D, 2] = np.asarray(g2, f)
    bn[:D, 3] = np.asarray(be2, f)
    bn[:D, 4] = np.asarray(g3, f)
    bn[:D, 5] = np.asarray(be3, f)

    in_maps = []
    for c in range(NCORES):
        sl = slice(c * DP, (c + 1) * DP)
        in_maps.append({
            "patches": patches,
            "w1t": np.ascontiguousarray(w1p[sl].T),   # [25, DP]
            "w2": np.ascontiguousarray(w2p[sl]),
            "w3": np.ascontiguousarray(w3p[sl]),
            "bn": np.ascontiguousarray(bn[sl]),
            "wct": np.ascontiguousarray(wcp[sl]),
        })
    return in_maps


def kernel(**inputs) -> np.ndarray:
    from concourse.bass_utils import run_bass_kernel_spmd
    nc = get_nc()
    in_maps = prep_inputs(**inputs)
    res = run_bass_kernel_spmd(nc, in_maps, list(range(NCORES)))
    return np.asarray(res.results[0]["sims"], np.float32)


# revision 9
# speedup vs baseline: 1.0830x; 1.0830x over previous
"""Trainium2 Bass kernel for nn_ConvHDC (binary HDC conv encoder + classifier).

Strategy (per sharding hint): shard the D=10000 hyperdimension across the 8
NeuronCores (pad to 10240 -> 1280 channels/core = 10 tiles of 128 partitions).
Everything after conv1 is depthwise / per-channel, so the whole network runs
locally per core; only the final [16,10] partial similarity matrix is
AllReduce-summed across cores.

Per-core pipeline (per 128-channel tile):
  conv1  : TensorE matmul, patches [25,2705] x sign(W1) [25,128] -> PSUM
           (last rhs column is the column-sum of patches => per-channel SUM
            of conv1 outputs lands in PSUM for free - used for BN mean)
  BN1    : sum from matmul; sum-of-squares via chained tensor_tensor_reduce
           (DVE); binarize folded into one ACT pass: sign(x + bias) with
           bias = (beta-0.5)/gamma*sqrt(var+eps) - mean  (valid for gamma>0)
  conv2  : depthwise 3x3/s2 as 9 accumulating TensorE matmuls with
           diagonalized per-channel weights (built by one gpsimd
           affine_select per tile), bf16 (exact: h1b in {-1,0,1})
  BN2    : bn_stats/bn_aggr (equal-size chunks keep bn_aggr exact)
  conv3  : depthwise 6x6 -> 1x1 on DVE (broadcast multiply + reduce)
  BN3    : bn_stats/bn_aggr
  final  : [16,10] partial sims via 10 accumulating matmuls (bf16, exact),
           scale 1/sqrt(D), DRAM bounce, AllReduce over 8 cores.
"""

import sys

if "/opt/trn_rl_repo" not in sys.path:
    sys.path.insert(0, "/opt/trn_rl_repo")

import numpy as np
from numpy.lib.stride_tricks import sliding_window_view

from concourse import bacc, tile, mybir
from concourse import bass as bass_mod

F32 = mybir.dt.float32
BF16 = mybir.dt.bfloat16
ALU = mybir.AluOpType
ACTF = mybir.ActivationFunctionType

NCORES = 8
D = 10000
DPAD = 10240
DP = DPAD // NCORES          # 1280 channels per core
DT = DP // 128               # 10 tiles of 128 channels
B = 16
EPS = 1e-5

H1 = 13                      # conv1 output spatial
N1 = B * H1 * H1             # 2704
CH1 = [452, 452, 452, 452, 452, 444]   # conv1 chunk sizes (last +1 sum col)
H2 = 6
N2 = B * H2 * H2             # 576
NB2 = 8                      # batches per conv2 group (2 groups of 288 cols)
G2 = N2 // 2                 # 288

# conv1 matmul input dtype: float32 (exact, 4 cy/row) or float32r (1 cy/row,
# precision must be validated on hardware before enabling)
CONV1_DTYPE = "float32"

_CACHE = {}


def _build_bass():
    """Build + compile the 8-core Bass program. Returns the Bacc object."""
    nc = bacc.Bacc("TRN2", target_bir_lowering=False, debug=False,
                   num_devices=NCORES)

    ph_d = nc.dram_tensor("ph", [25, N1 + 1], BF16, kind="ExternalInput").ap()
    pl_d = nc.dram_tensor("pl", [25, N1 + 1], BF16, kind="ExternalInput").ap()
    pll_d = nc.dram_tensor("pll", [25, N1 + 1], BF16, kind="ExternalInput").ap()
    w1t_d = nc.dram_tensor("w1t", [25, DP], F32, kind="ExternalInput").ap()
    w2_d = nc.dram_tensor("w2", [DP, 9], F32, kind="ExternalInput").ap()
    w3_d = nc.dram_tensor("w3", [DP, 36], F32, kind="ExternalInput").ap()
    bn_d = nc.dram_tensor("bn", [DP, 6], F32, kind="ExternalInput").ap()
    wct_d = nc.dram_tensor("wct", [DP, 10], F32, kind="ExternalInput").ap()
    out_d = nc.dram_tensor("sims", [B, 10], F32, kind="ExternalOutput").ap()

    with tile.TileContext(nc) as tc:
        with (
            tc.tile_pool(name="const", bufs=1) as const,
            tc.tile_pool(name="work", bufs=3) as work,
            tc.tile_pool(name="stat", bufs=2) as stat,
            tc.tile_pool(name="psum1", bufs=1, space="PSUM") as psum1,
            tc.tile_pool(name="psum2", bufs=1, space="PSUM") as psum2,
            tc.tile_pool(name="dram", bufs=1, space="DRAM") as dram,
        ):
            # ---------------- constants / weights ----------------
            # conv1 moving operand: 3-term bf16 decomposition of the patch
            # matrix (hi+lo+lolo ~ 24 significant bits = fp32-class precision)
            pparts = []
            for nm, pd_ in (("ph", ph_d), ("pl", pl_d), ("pll", pll_d)):
                pt_ = const.tile([25, N1 + 1], BF16, tag=f"p_{nm}", name=f"p_{nm}")
                nc.sync.dma_start(out=pt_[:], in_=pd_[:])
                pparts.append(pt_)

            w1raw = const.tile([25, DP], F32)
            nc.sync.dma_start(out=w1raw[:], in_=w1t_d[:])
            w1s = const.tile([25, DP], BF16)
            nc.scalar.sign(w1s[:], w1raw[:])

            w2raw = const.tile([128, DT, 9], F32)
            nc.sync.dma_start(
                out=w2raw[:], in_=w2_d.rearrange("(t p) k -> p t k", t=DT, p=128))
            w2s = const.tile([128, DT, 9], BF16)
            nc.scalar.sign(w2s[:], w2raw[:])

            w3t = const.tile([128, DT, 36], F32)
            nc.sync.dma_start(
                out=w3t[:], in_=w3_d.rearrange("(t p) k -> p t k", t=DT, p=128))

            bnt = const.tile([128, DT, 6], F32)
            nc.sync.dma_start(
                out=bnt[:], in_=bn_d.rearrange("(t p) c -> p t c", t=DT, p=128))

            wcraw = const.tile([128, DT, 10], F32)
            nc.sync.dma_start(
                out=wcraw[:], in_=wct_d.rearrange("(t p) c -> p t c", t=DT, p=128))
            wcs = const.tile([128, DT, 10], BF16)
            nc.scalar.sign(wcs[:], wcraw[:])

            epsc = const.tile([128, 1], F32)
            nc.vector.memset(epsc[:], EPS)

            # beMrg[:, t, L] = (beta_L - 0.5) / gamma_L   (gamma > 0 assumed)
            rg = const.tile([128, DT, 3], F32)
            nc.vector.reciprocal(rg[:], bnt[:, :, 0::2])
            beMrg = const.tile([128, DT, 3], F32)
            nc.vector.tensor_scalar(beMrg[:], bnt[:, :, 1::2], -0.5, None, ALU.add)
            nc.vector.tensor_tensor(beMrg[:], beMrg[:], rg[:], ALU.mult)

            h3b_all = const.tile([128, DT, B], BF16)

            # ---------------- per-tile pipeline ----------------
            for t in range(DT):
                lhs1 = w1s[:, t * 128:(t + 1) * 128]

                h1b = work.tile([128, N1], BF16, tag="h1b")
                st1 = stat.tile([128, 6, 6], F32, tag="st1")

                p1 = []
                off = 0
                for ci, csz in enumerate(CH1):
                    w = csz + 1 if ci == 5 else csz
                    pt = psum1.tile([128, w], F32, tag=f"c1_{ci}")
                    for pi, part in enumerate(pparts):
                        nc.tensor.matmul(pt[:], lhsT=lhs1,
                                         rhs=part[:, off:off + w],
                                         start=(pi == 0), stop=(pi == 2))
                    nc.vector.bn_stats(st1[:, ci, :], pt[:, 0:csz])
                    p1.append(pt)
                    off += csz

                # BN1: mean from matmul sum column; var from exact combine of
                # bn_stats sub-group stats (S2 = sum nv_i + sum n_i*m_i^2,
                # group counts are compile-time constants: 226 for the five
                # 452-chunks' even/odd halves, 222 for the last 444-chunk).
                mean1 = stat.tile([128, 1], F32, tag="mean1")
                nc.vector.tensor_scalar(
                    mean1[:], p1[5][:, 444:445], 1.0 / N1, None, ALU.mult)
                m2 = stat.tile([128, 1], F32, tag="m2")
                nc.vector.tensor_tensor(m2[:], mean1[:], mean1[:], ALU.mult)
                mm = stat.tile([128, 6, 2], F32, tag="mm")
                nc.vector.tensor_tensor(
                    mm[:], st1[:, :, 1::3], st1[:, :, 1::3], ALU.mult)
                s226 = stat.tile([128, 1], F32, tag="s226")
                nc.vector.tensor_reduce(
                    s226[:], mm[:, 0:5, :].rearrange("p a b -> p (a b)"),
                    mybir.AxisListType.X, ALU.add)
                s222 = stat.tile([128, 1], F32, tag="s222")
                nc.vector.tensor_reduce(
                    s222[:], mm[:, 5, :], mybir.AxisListType.X, ALU.add)
                snv = stat.tile([128, 1], F32, tag="snv")
                nc.vector.tensor_reduce(
                    snv[:], st1[:, :, 2::3].rearrange("p a b -> p (a b)"),
                    mybir.AxisListType.X, ALU.add)
                u1 = stat.tile([128, 1], F32, tag="u1")
                nc.vector.tensor_scalar(
                    u1[:], s226[:], 226.0, snv[:], ALU.mult, ALU.add)
                ssqt = stat.tile([128, 1], F32, tag="ssqt")
                nc.vector.tensor_scalar(
                    ssqt[:], s222[:], 222.0, u1[:], ALU.mult, ALU.add)
                ve1 = stat.tile([128, 1], F32, tag="ve1")
                nc.vector.tensor_scalar(
                    ve1[:], ssqt[:], 1.0 / N1, m2[:], ALU.mult, ALU.subtract)
                sq1 = stat.tile([128, 1], F32, tag="sq1")
                nc.scalar.activation(sq1[:], ve1[:], ACTF.Sqrt, bias=epsc[:], scale=1.0)
                bias1 = stat.tile([128, 1], F32, tag="bias1")
                nc.vector.tensor_scalar(
                    bias1[:], sq1[:], beMrg[:, t, 0:1], mean1[:],
                    ALU.mult, ALU.subtract)

                off = 0
                for ci, csz in enumerate(CH1):
                    nc.scalar.activation(
                        h1b[:, off:off + csz], p1[ci][:, 0:csz], ACTF.Sign,
                        bias=bias1[:], scale=1.0)
                    off += csz

                # ---- conv2: diagonalized depthwise 3x3 stride 2 ----
                diag9 = work.tile([128, 9, 128], BF16, tag="diag9")
                nc.gpsimd.affine_select(
                    out=diag9[:],
                    in_=w2s[:, t, :].unsqueeze(2).broadcast_to([128, 9, 128]),
                    pattern=[[0, 9], [1, 128]], base=0, channel_multiplier=-1,
                    compare_op=ALU.is_equal, fill=0.0)

                h1b4 = h1b[:].rearrange("p (b h w) -> p b h w", b=B, h=H1, w=H1)
                st2 = stat.tile([128, 2, 6], F32, tag="st2")
                p2 = [psum2.tile([128, G2], F32, tag=f"c2_{g}", name=f"p2_{g}") for g in range(2)]
                for k in range(9):
                    kh, kw = k // 3, k % 3
                    for g in range(2):
                        rhs = h1b4[:, g * NB2:(g + 1) * NB2,
                                   kh:kh + 11:2, kw:kw + 11:2]
                        nc.tensor.matmul(p2[g][:], lhsT=diag9[:, k, :], rhs=rhs,
                                         start=(k == 0), stop=(k == 8))
                for g in range(2):
                    nc.vector.bn_stats(st2[:, g, :], p2[g][:])

                mv2 = stat.tile([128, 2], F32, tag="mv2")
                nc.vector.bn_aggr(mv2[:], st2[:])
                sq2 = stat.tile([128, 1], F32, tag="sq2")
                nc.scalar.activation(sq2[:], mv2[:, 1:2], ACTF.Sqrt,
                                     bias=epsc[:], scale=1.0)
                bias2 = stat.tile([128, 1], F32, tag="bias2")
                nc.vector.tensor_scalar(
                    bias2[:], sq2[:], beMrg[:, t, 1:2], mv2[:, 0:1],
                    ALU.mult, ALU.subtract)

                h2b = work.tile([128, N2], BF16, tag="h2b")
                for g in range(2):
                    nc.scalar.activation(
                        h2b[:, g * G2:(g + 1) * G2], p2[g][:], ACTF.Sign,
                        bias=bias2[:], scale=1.0)

                # ---- conv3: depthwise 6x6 -> 1 (DVE) ----
                tmp3 = work.tile([128, B, 36], F32, tag="tmp3")
                h2b3 = h2b[:].rearrange("p (b s) -> p b s", b=B, s=36)
                w3b = w3t[:, t, :].unsqueeze(1).broadcast_to([128, B, 36])
                nc.gpsimd.tensor_tensor(tmp3[:], h2b3, w3b, ALU.mult)
                h3pre = work.tile([128, B], F32, tag="h3pre")
                nc.vector.tensor_reduce(h3pre[:], tmp3[:], mybir.AxisListType.X,
                                        ALU.add)

                st3 = stat.tile([128, 6], F32, tag="st3")
                nc.vector.bn_stats(st3[:], h3pre[:])
                mv3 = stat.tile([128, 2], F32, tag="mv3")
                nc.vector.bn_aggr(mv3[:], st3[:])
                sq3 = stat.tile([128, 1], F32, tag="sq3")
                nc.scalar.activation(sq3[:], mv3[:, 1:2], ACTF.Sqrt,
                                     bias=epsc[:], scale=1.0)
                bias3 = stat.tile([128, 1], F32, tag="bias3")
                nc.vector.tensor_scalar(
                    bias3[:], sq3[:], beMrg[:, t, 2:3], mv3[:, 0:1],
                    ALU.mult, ALU.subtract)
                nc.scalar.activation(h3b_all[:, t, :], h3pre[:], ACTF.Sign,
                                     bias=bias3[:], scale=1.0)

            # ---------------- classifier + AllReduce ----------------
            psims = psum2.tile([B, 10], F32, tag="c2_0")
            for t in range(DT):
                nc.tensor.matmul(psims[:], lhsT=h3b_all[:, t, :],
                                 rhs=wcs[:, t, :],
                                 start=(t == 0), stop=(t == DT - 1))
            sims_sb = stat.tile([B, 10], F32, tag="sims_sb")
            nc.scalar.mul(sims_sb[:], psims[:], 1.0 / np.sqrt(np.float32(D)))

            in_bounce = dram.tile([B, 10], F32)
            out_bounce = dram.tile([B, 10], F32)
            nc.sync.dma_start(out=in_bounce[:], in_=sims_sb[:])
            nc.gpsimd.collective_compute(
                "AllReduce", ALU.add,
                replica_groups=[list(range(NCORES))],
                ins=[in_bounce.opt()], outs=[out_bounce.opt()])
            nc.sync.dma_start(out=out_d[:], in_=out_bounce[:])

    nc.compile()
    return nc


def get_nc():
    if "nc" not in _CACHE:
        _CACHE["nc"] = _build_bass()
    return _CACHE["nc"]


def prep_inputs(x, W1, b1, g1, be1, W2, b2, g2, be2, W3, b3, g3, be3, Wc):
    """Host-side layout/sharding prep (layout only, no model math).

    Conv biases b1/b2/b3 are dropped: training-mode BN is invariant to a
    per-channel additive constant before normalization.
    """
    f = np.float32

    import ml_dtypes
    bf = ml_dtypes.bfloat16

    xp = np.zeros((B, 30, 30), f)
    xp[:, 1:29, 1:29] = np.asarray(x, f)[:, 0]
    win = sliding_window_view(xp, (5, 5), axis=(1, 2))[:, ::2, ::2]  # B,13,13,5,5
    patches = np.ascontiguousarray(
        win.transpose(3, 4, 0, 1, 2).reshape(25, N1)).astype(f)
    pext = np.zeros((25, N1 + 1), f)
    pext[:, :N1] = patches
    pext[:, N1] = patches.sum(1, dtype=np.float64).astype(f)
    ph = pext.astype(bf)
    pl = (pext - ph.astype(f)).astype(bf)
    pll = (pext - ph.astype(f) - pl.astype(f)).astype(bf)

    def padrows(a, width, fill=0.0):
        out = np.full((DPAD, width), fill, f)
        out[:D] = np.asarray(a, f).reshape(D, width)
        return out

    w1p = padrows(W1, 25)               # [DPAD, 25]
    w2p = padrows(W2, 9)
    w3p = padrows(W3, 36)
    wcp = padrows(np.asarray(Wc, f).T, 10)
    bn = np.zeros((DPAD, 6), f)
    bn[:, 0::2] = 1.0                    # pad gamma=1
    bn[:D, 0] = np.asarray(g1, f)
    bn[:D, 1] = np.asarray(be1, f)
    bn[:D, 2] = np.asarray(g2, f)
    bn[:D, 3] = np.asarray(be2, f)
    bn[:D, 4] = np.asarray(g3, f)
    bn[:D, 5] = np.asarray(be3, f)

    in_maps = []
    for c in range(NCORES):
        sl = slice(c * DP, (c + 1) * DP)
        in_maps.append({
            "ph": ph, "pl": pl, "pll": pll,
            "w1t": np.ascontiguousarray(w1p[sl].T),   # [25, DP]
            "w2": np.ascontiguousarray(w2p[sl]),
            "w3": np.ascontiguousarray(w3p[sl]),
            "bn": np.ascontiguousarray(bn[sl]),
            "wct": np.ascontiguousarray(wcp[sl]),
        })
    return in_maps


def kernel(**inputs) -> np.ndarray:
    from concourse.bass_utils import run_bass_kernel_spmd
    nc = get_nc()
    in_maps = prep_inputs(**inputs)
    res = run_bass_kernel_spmd(nc, in_maps, list(range(NCORES)))
    return np.asarray(res.results[0]["sims"], np.float32)
